# revision 13
# baseline (speedup 1.0000x reference)
"""Cross-modal triplet loss (margin ranking on hardest pos/neg pairs) on 8 trn2 NeuronCores.

2D sharding: modal1 rows split 4 ways x modal2 columns split 2 ways (core id =
(row_shard, col_shard)). Each core computes its 1024x2048 slab of the pairwise
squared-distance matrix with one fp32 PSUM accumulation group per (128-row
m-tile, 512-col chunk):

    psum[m, j] = dot(m1q[m], m2q[j]) - sq2q[j]/2 - (BIG/2) * mask[m, j]

Inputs are quantized to fp8 e4m3 and the matmuls run in DoubleRow perf mode
(two 128-deep k-tiles per pass, ~1.5-2x the bf16 rate): 8 DoubleRow passes over
the 16 data k-tiles plus one plain fp8 pass for the augmented k-tile. The
augmented tile carries the same-identity mask (64 ids -> onehot rows, lhs coeff
-128 x rhs 64 = -8192 per same-id pair) and -sq2/2 as a 4-term radix expansion
(lhs coeffs 128, 8, 0.25, 2^-6 x fp8 rhs rows; residual < 2e-3).

All layout work happens on the host: m1/m2 are quantized and pre-transposed into
k-tile-major SBUF layouts ([P, KT, cols]) so DoubleRow's 3D [128, 2, N] access
patterns are direct slices. DMAs are packed into a handful of large transfers
split across the two HWDGE queues (sync=SP, scalar=Act) in consumption order,
with the first k-tiles in small lead pieces so the PE starts early. The first
chunk's matmuls are emitted k-outer across all 8 open PSUM groups so they
consume k-tile pieces as they land.

Row-wise min of psum gives the hardest-positive (as -2*psum it is the max);
row-wise max gives the hardest-negative (vector engine; GpSimd has no PSUM
port). The device emits only per-row partial min/max [P, 2*MT]; the host
combines the two column shards, adds the row norm sq1, takes sqrt (monotone)
and computes the hinge loss / precision over all 4096 rows.

Numerics: inputs are ~N(0,1) and the per-row hardest-pos/neg gap is >4.2. With
sq1/sq2 computed on the host from the *quantized* vectors, the kernel computes
the exact distance geometry of the quantized point set; host-simulated rel err
vs the fp32 reference is 7e-4 (gate 2e-2), precision exactly 0 either way.
"""

import functools

import numpy as np
import ml_dtypes

import concourse.bass as bass
import concourse.mybir as mybir
import concourse.tile as tile
from concourse import bacc
from concourse.bass_utils import run_bass_kernel_spmd

F32 = mybir.dt.float32
F8 = mybir.dt.float8e4
OP = mybir.AluOpType
AX = mybir.AxisListType.X
DR = mybir.MatmulPerfMode.DoubleRow

NP_F8 = ml_dtypes.float8_e4m3

N, D, NIDS, P = 4096, 2048, 64, 128
NCORES = 8
RS, CS = 4, 2         # row shards x col shards
NR = N // RS          # 1024 modal1 rows per core
NC2 = N // CS         # 2048 modal2 cols per core
MT = NR // P          # 8 m-tiles per core
NKD = D // P          # 16 data k-tiles
KT = NKD + 1          # + 1 aug k-tile
NDR = NKD // 2        # 8 DoubleRow passes
CHUNK = 512           # modal2 cols per PSUM group (one fp32 PSUM bank)
NJC = NC2 // CHUNK    # 4 chunks per core
PAIR = 2 * CHUNK      # two chunks share one packed rhs block
NPAIR = NC2 // PAIR   # 2
BIG = 16384.0         # > max (dist_sq - sq1) spread
EPS = 1e-12

# -sq2/2 radix expansion: v = sum_t COEFFS[t] * fp8(rem_t / COEFFS[t])
COEFFS = (128.0, 8.0, 0.25, 2.0 ** -6)

# k-tile DMA pieces: small lead pieces so the PE starts early, then bulk.
PIECES = [(0, 2), (2, 5), (7, 10)]  # (start kt, n kt)


def _build() -> bass.Bass:
    nc = bacc.Bacc(num_swdge_queues=4)
    lhs_d = nc.dram_tensor("lhs", [P, KT, NR], F8, kind="ExternalInput")
    rhs_d = nc.dram_tensor("rhs", [NPAIR, P, KT, PAIR], F8, kind="ExternalInput")
    out_d = nc.dram_tensor("out", [P, 2 * MT], F32, kind="ExternalOutput")

    with tile.TileContext(nc) as tc:
        with (
            tc.tile_pool(name="lhs", bufs=1) as lhsp,
            tc.tile_pool(name="rhs", bufs=1) as rhsp,
            tc.tile_pool(name="stat", bufs=2 * MT + 4) as statp,
            tc.tile_pool(name="psD", bufs=8, space=bass.MemorySpace.PSUM) as psD,
        ):
            # --- DMA program. Per-queue FIFO order is consumption order:
            # sync:   lhs pieces, rhs pair1, (out at the end)
            # scalar: rhs pair0 pieces
            lhs_t = lhsp.tile([P, KT, NR], F8, tag="lhs", name="lhs_t")
            for k0, nk in PIECES:
                nc.sync.dma_start(
                    lhs_t[:, k0 : k0 + nk, :], lhs_d[:, k0 : k0 + nk, :]
                )

            rhs_t = []
            for pr in range(NPAIR):
                t = rhsp.tile([P, KT, PAIR], F8, tag=f"rhs{pr}", name=f"rhs{pr}")
                rhs_t.append(t)
            for k0, nk in PIECES:  # pair0 in pieces on scalar queue
                nc.scalar.dma_start(
                    rhs_t[0][:, k0 : k0 + nk, :], rhs_d[0, :, k0 : k0 + nk, :]
                )
            nc.sync.dma_start(rhs_t[1][:, :, :], rhs_d[1, :, :, :])

            minb = [statp.tile([P, NJC], F32, tag="stat", name=f"minb{i}") for i in range(MT)]
            maxb = [statp.tile([P, NJC], F32, tag="stat", name=f"maxb{i}") for i in range(MT)]

            def mm_dr(pd, ktp, jc, mt):
                pr, half = jc // 2, jc % 2
                nc.tensor.matmul(
                    pd[:],
                    lhs_t[:, 2 * ktp : 2 * ktp + 2, mt * P : (mt + 1) * P],
                    rhs_t[pr][:, 2 * ktp : 2 * ktp + 2,
                              half * CHUNK : (half + 1) * CHUNK],
                    start=(ktp == 0), stop=False,
                    perf_mode=DR,
                )

            def mm_aug(pd, jc, mt):
                pr, half = jc // 2, jc % 2
                nc.tensor.matmul(
                    pd[:],
                    lhs_t[:, NKD, mt * P : (mt + 1) * P],
                    rhs_t[pr][:, NKD, half * CHUNK : (half + 1) * CHUNK],
                    start=False, stop=True,
                )

            def reduces(pd, jc, mt):
                nc.vector.tensor_reduce(
                    minb[mt][:, jc : jc + 1], pd[:], AX, OP.min
                )
                nc.vector.tensor_reduce(
                    maxb[mt][:, jc : jc + 1], pd[:], AX, OP.max
                )

            # --- chunk 0: k-outer over all 8 open groups (consume pieces as
            # they land); groups mt in 0..7
            p0 = {}
            for mt in range(MT):
                p0[mt] = psD.tile([P, CHUNK], F32, tag="psD", name=f"p0_{mt}")
            for ktp in range(NDR):
                for mt in range(MT):
                    mm_dr(p0[mt], ktp, 0, mt)
            for mt in range(MT):
                mm_aug(p0[mt], 0, mt)
            for mt in range(MT):
                reduces(p0[mt], 0, mt)

            # --- chunks 1-3: group-major, banks rotate through the pool
            for jc in range(1, NJC):
                for mt in range(MT):
                    pd = psD.tile([P, CHUNK], F32, tag="psD")
                    for ktp in range(NDR):
                        mm_dr(pd, ktp, jc, mt)
                    mm_aug(pd, jc, mt)
                    reduces(pd, jc, mt)

            # ---- cross-chunk combine; everything else happens on the host
            lp = statp.tile([P, 2 * MT], F32, tag="fin")
            for mt in range(MT):
                nc.vector.tensor_reduce(
                    lp[:, mt : mt + 1], minb[mt][:], AX, OP.min
                )
                nc.vector.tensor_reduce(
                    lp[:, MT + mt : MT + mt + 1], maxb[mt][:], AX, OP.max
                )
            nc.sync.dma_start(out_d[:, :], lp[:])

    nc.finalize()
    return nc


@functools.lru_cache(maxsize=1)
def _get_program() -> bass.Bass:
    return _build()


def _prep_host(m1, m2, targets):
    """Quantize to fp8 e4m3 and pack into the k-tile-major DMA layouts."""
    m1q = m1.astype(NP_F8)
    m2q = m2.astype(NP_F8)
    sq1 = (m1q.astype(np.float64) ** 2).sum(axis=1)  # [N]
    sq2 = (m2q.astype(np.float64) ** 2).sum(axis=1)  # [N]
    tgt = np.asarray(targets).astype(np.int64)

    onehot = tgt[None, :] == np.arange(NIDS, dtype=np.int64)[:, None]  # [64, N]

    # rhs aug rows: 64*onehot (mask, pairs with lhs coeff -128) + radix rows
    # encoding -sq2/2 (pair with lhs coeffs COEFFS).
    raug = np.zeros((P, N), dtype=NP_F8)
    raug[:NIDS] = (64.0 * onehot.astype(np.float32)).astype(NP_F8)
    rem = -0.5 * sq2
    for t, c in enumerate(COEFFS):
        q = (rem / c).astype(np.float32).astype(NP_F8)
        raug[NIDS + t] = q
        rem = rem - c * q.astype(np.float64)

    # rhs_aug: [17, 128, N] fp8 = m2T (16 k-tiles) + aug k-tile
    rhs_aug = np.zeros((KT, P, N), dtype=NP_F8)
    rhs_aug[:NKD] = np.ascontiguousarray(m2q.T).reshape(NKD, P, N)
    rhs_aug[NKD] = raug
    # pack [17,128,4,1024] -> per col shard [2, 128, 17, 1024]
    rp = rhs_aug.reshape(KT, P, CS * NPAIR, PAIR).transpose(2, 1, 0, 3)
    rhs_all = [np.ascontiguousarray(rp[cs * NPAIR : (cs + 1) * NPAIR])
               for cs in range(CS)]

    lhs_all = []
    for rs in range(RS):
        sl = slice(rs * NR, (rs + 1) * NR)
        lhs_aug = np.zeros((KT, P, NR), dtype=NP_F8)
        lhs_aug[:NKD] = np.ascontiguousarray(m1q[sl].T).reshape(NKD, P, NR)
        laug = np.zeros((P, NR), dtype=np.float32)
        laug[:NIDS] = -128.0 * onehot[:, sl].astype(np.float32)
        for t, cf in enumerate(COEFFS):
            laug[NIDS + t] = cf
        lhs_aug[NKD] = laug.astype(NP_F8)
        lhs_all.append(np.ascontiguousarray(lhs_aug.transpose(1, 0, 2)))
    return lhs_all, rhs_all, sq1


def run(modal1_inputs, modal2_inputs, targets, margin, trace=False):
    m1 = np.ascontiguousarray(np.asarray(modal1_inputs, dtype=np.float32))
    m2 = np.ascontiguousarray(np.asarray(modal2_inputs, dtype=np.float32))
    lhs_all, rhs_all, sq1 = _prep_host(m1, m2, targets)
    nc = _get_program()
    in_maps = [
        {"lhs": lhs_all[c // CS], "rhs": rhs_all[c % CS]} for c in range(NCORES)
    ]
    res = run_bass_kernel_spmd(nc, in_maps, list(range(NCORES)), trace=trace)

    # host finale: combine col shards, add norms, sqrt, hinge
    pmin = np.empty(N)
    pmax = np.empty(N)
    for rs in range(RS):
        outs = [np.asarray(res.results[rs * CS + cs]["out"], dtype=np.float64)
                for cs in range(CS)]
        mn = np.minimum(outs[0][:, :MT], outs[1][:, :MT])  # [P, MT]
        mx = np.maximum(outs[0][:, MT:], outs[1][:, MT:])
        sl = slice(rs * NR, (rs + 1) * NR)
        pmin[sl] = mn.T.reshape(NR)
        pmax[sl] = mx.T.reshape(NR)
    ap = np.sqrt(np.maximum(-2.0 * pmin + sq1 - BIG, EPS))
    an = np.sqrt(np.maximum(-2.0 * pmax + sq1, EPS))
    loss = np.float32(np.maximum(ap - an + float(margin), 0.0).mean())
    prec = np.float32((an > ap).mean())
    return (loss, prec), res


def kernel(modal1_inputs, modal2_inputs, targets, margin):
    (loss, prec), _ = run(modal1_inputs, modal2_inputs, targets, margin)
    return loss, prec


# revision 14
# speedup vs baseline: 1.0602x; 1.0602x over previous
"""Cross-modal triplet loss (margin ranking on hardest pos/neg pairs) on 8 trn2 NeuronCores.

Strategy (per sharding hint): shard rows of modal1 across the 8 cores (512 rows
each); replicate modal2 and targets. Each core computes its 512x4096 slab of the
pairwise squared-distance matrix with one fp32 PSUM accumulation group per
(128-row m-tile, 512-col chunk):

    psum[m, j] = dot(m1q[m], m2q[j]) - sq2q[j]/2 - (BIG/2) * mask[m, j]

Inputs are quantized to fp8 e4m3 and the matmuls run in DoubleRow perf mode
(two 128-deep k-tiles per pass, ~1.5-2x the bf16 rate): 8 DoubleRow passes over
the 16 data k-tiles plus one plain fp8 pass for the augmented k-tile. The
augmented tile carries the same-identity mask (64 ids -> onehot rows, lhs coeff
-128 x rhs 64 = -8192 per same-id pair) and -sq2/2 as a 4-term radix expansion
(lhs coeffs 128, 8, 0.25, 2^-6 x fp8 rhs rows; residual < 2e-3).

All layout work happens on the host: m1/m2 are quantized and pre-transposed into
k-tile-major SBUF layouts ([P, KT, cols]) so DoubleRow's 3D [128, 2, N] access
patterns are direct slices. DMAs are packed into a handful of large transfers
split across the two HWDGE queues (sync=SP, scalar=Act) in consumption order,
with the first k-tiles in small lead pieces so the PE starts early. The first
chunk-pair's matmuls are emitted k-outer across all 8 open PSUM groups so they
consume k-tile pieces as they land.

Row-wise min of psum gives the hardest-positive (as -2*psum it is the max);
row-wise max gives the hardest-negative. Both reduces run on the vector engine
(GpSimd has no PSUM port). The m1-row norm sq1q[m] is constant per row and is
added after the reduction. sqrt only on the final per-row reductions (monotone).
Per-row loss/precision terms are column-packed to [128, 8] and DMA'd out; the
host does the final 128-row sum.

Numerics: inputs are ~N(0,1) and the per-row hardest-pos/neg gap is >4.2. With
sq1/sq2 computed on the host from the *quantized* vectors, the kernel computes
the exact distance geometry of the quantized point set; host-simulated rel err
vs the fp32 reference is 7e-4 (gate 2e-2), precision exactly 0 either way.
"""

import functools

import numpy as np
import ml_dtypes

import concourse.bass as bass
import concourse.mybir as mybir
import concourse.tile as tile
from concourse import bacc
from concourse.bass_utils import run_bass_kernel_spmd

F32 = mybir.dt.float32
F8 = mybir.dt.float8e4
OP = mybir.AluOpType
AF = mybir.ActivationFunctionType
AX = mybir.AxisListType.X
DR = mybir.MatmulPerfMode.DoubleRow

NP_F8 = ml_dtypes.float8_e4m3

N, D, NIDS, P = 4096, 2048, 64, 128
NCORES = 8
SH = N // NCORES      # 512 rows of modal1 per core
MT = SH // P          # 4 m-tiles per core
NKD = D // P          # 16 data k-tiles
KT = NKD + 1          # + 1 aug k-tile
NDR = NKD // 2        # 8 DoubleRow passes
CHUNK = 512           # modal2 cols per PSUM group (one fp32 PSUM bank)
NJC = N // CHUNK      # 8 chunks
PAIR = 2 * CHUNK      # two chunks share one packed rhs block
NPAIR = N // PAIR     # 4
BIG = 16384.0         # > max (dist_sq - sq1) spread; exact in fp8 lhs*rhs split
EPS = 1e-12

# -sq2/2 radix expansion: v = sum_t COEFFS[t] * fp8(rem_t / COEFFS[t])
COEFFS = (128.0, 8.0, 0.25, 2.0 ** -6)
NAUG = NIDS + len(COEFFS)  # used aug rows

# k-tile DMA pieces: small lead pieces so the PE starts early, then bulk.
PIECES = [(0, 2), (2, 5), (7, 10)]  # (start kt, n kt)


def _build() -> bass.Bass:
    nc = bacc.Bacc(num_swdge_queues=4)
    lhs_d = nc.dram_tensor("lhs", [P, KT, SH], F8, kind="ExternalInput")
    rhs_d = nc.dram_tensor("rhs", [NPAIR, P, KT, PAIR], F8, kind="ExternalInput")
    out_d = nc.dram_tensor("out", [P, 2 * MT], F32, kind="ExternalOutput")

    with tile.TileContext(nc) as tc:
        with (
            tc.tile_pool(name="lhs", bufs=1) as lhsp,
            tc.tile_pool(name="rhs", bufs=1) as rhsp,
            tc.tile_pool(name="stat", bufs=2 * MT + 16) as statp,
            tc.tile_pool(name="psD", bufs=8, space=bass.MemorySpace.PSUM) as psD,
        ):
            # --- DMA program. Per-queue FIFO order is consumption order:
            # sync:   lhs pieces, rhs pair1, rhs pair3, sq1, sq1b
            # scalar: rhs pair0 pieces, rhs pair2
            lhs_t = lhsp.tile([P, KT, SH], F8, tag="lhs", name="lhs_t")
            for k0, nk in PIECES:
                nc.sync.dma_start(
                    lhs_t[:, k0 : k0 + nk, :], lhs_d[:, k0 : k0 + nk, :]
                )

            rhs_t = []
            for pr in range(NPAIR):
                t = rhsp.tile([P, KT, PAIR], F8, tag=f"rhs{pr}", name=f"rhs{pr}")
                rhs_t.append(t)
            for k0, nk in PIECES:  # pair0 in pieces on scalar queue
                nc.scalar.dma_start(
                    rhs_t[0][:, k0 : k0 + nk, :], rhs_d[0, :, k0 : k0 + nk, :]
                )
            nc.sync.dma_start(rhs_t[1][:, :, :], rhs_d[1, :, :, :])
            nc.scalar.dma_start(rhs_t[2][:, :, :], rhs_d[2, :, :, :])
            nc.sync.dma_start(rhs_t[3][:, :, :], rhs_d[3, :, :, :])

            minb = [statp.tile([P, NJC], F32, tag="stat", name=f"minb{i}") for i in range(MT)]
            maxb = [statp.tile([P, NJC], F32, tag="stat", name=f"maxb{i}") for i in range(MT)]

            def mm_dr(pd, ktp, jc, mt):
                pr, half = jc // 2, jc % 2
                nc.tensor.matmul(
                    pd[:],
                    lhs_t[:, 2 * ktp : 2 * ktp + 2, mt * P : (mt + 1) * P],
                    rhs_t[pr][:, 2 * ktp : 2 * ktp + 2,
                              half * CHUNK : (half + 1) * CHUNK],
                    start=(ktp == 0), stop=False,
                    perf_mode=DR,
                )

            def mm_aug(pd, jc, mt):
                pr, half = jc // 2, jc % 2
                nc.tensor.matmul(
                    pd[:],
                    lhs_t[:, NKD, mt * P : (mt + 1) * P],
                    rhs_t[pr][:, NKD, half * CHUNK : (half + 1) * CHUNK],
                    start=False, stop=True,
                )

            def reduces(pd, jc, mt):
                nc.vector.tensor_reduce(
                    minb[mt][:, jc : jc + 1], pd[:], AX, OP.min
                )
                nc.vector.tensor_reduce(
                    maxb[mt][:, jc : jc + 1], pd[:], AX, OP.max
                )

            # --- pair 0: k-outer over all 8 open groups (consume pieces as
            # they land); groups (jc in {0,1}) x (mt in 0..3)
            p0 = {}
            for jc in (0, 1):
                for mt in range(MT):
                    p0[(jc, mt)] = psD.tile(
                        [P, CHUNK], F32, tag="psD", name=f"p0_{jc}_{mt}"
                    )
            for ktp in range(NDR):
                for jc in (0, 1):
                    for mt in range(MT):
                        mm_dr(p0[(jc, mt)], ktp, jc, mt)
            for jc in (0, 1):
                for mt in range(MT):
                    mm_aug(p0[(jc, mt)], jc, mt)
            for jc in (0, 1):
                for mt in range(MT):
                    reduces(p0[(jc, mt)], jc, mt)

            # --- pairs 1-3: group-major, banks rotate through the pool
            for jc in range(2, NJC):
                for mt in range(MT):
                    pd = psD.tile([P, CHUNK], F32, tag="psD")
                    for ktp in range(NDR):
                        mm_dr(pd, ktp, jc, mt)
                    mm_aug(pd, jc, mt)
                    reduces(pd, jc, mt)

            # ---- cross-chunk combine; everything else happens on the host
            lp = statp.tile([P, 2 * MT], F32, tag="fin")
            for mt in range(MT):
                nc.vector.tensor_reduce(
                    lp[:, mt : mt + 1], minb[mt][:], AX, OP.min
                )
                nc.vector.tensor_reduce(
                    lp[:, MT + mt : MT + mt + 1], maxb[mt][:], AX, OP.max
                )
            nc.sync.dma_start(out_d[:, :], lp[:])

    nc.finalize()
    return nc


@functools.lru_cache(maxsize=1)
def _get_program() -> bass.Bass:
    return _build()


def _prep_host(m1, m2, targets):
    """Quantize to fp8 e4m3 and pack into the k-tile-major DMA layouts."""
    m1q = m1.astype(NP_F8)
    m2q = m2.astype(NP_F8)
    sq1 = (m1q.astype(np.float64) ** 2).sum(axis=1)  # [N]
    sq2 = (m2q.astype(np.float64) ** 2).sum(axis=1)  # [N]
    tgt = np.asarray(targets).astype(np.int64)

    onehot = tgt[None, :] == np.arange(NIDS, dtype=np.int64)[:, None]  # [64, N]

    # rhs aug rows: 64*onehot (mask, pairs with lhs coeff -128) + radix rows
    # encoding -sq2/2 (pair with lhs coeffs COEFFS).
    raug = np.zeros((P, N), dtype=NP_F8)
    raug[:NIDS] = (64.0 * onehot.astype(np.float32)).astype(NP_F8)
    rem = -0.5 * sq2
    for t, c in enumerate(COEFFS):
        q = (rem / c).astype(np.float32).astype(NP_F8)
        raug[NIDS + t] = q
        rem = rem - c * q.astype(np.float64)

    # rhs_aug: [17, 128, N] fp8 = m2T (16 k-tiles) + aug k-tile
    rhs_aug = np.zeros((KT, P, N), dtype=NP_F8)
    rhs_aug[:NKD] = np.ascontiguousarray(m2q.T).reshape(NKD, P, N)
    rhs_aug[NKD] = raug
    # pack [17,128,4,1024] -> [4, 128, 17, 1024]
    rhs_p = np.ascontiguousarray(
        rhs_aug.reshape(KT, P, NPAIR, PAIR).transpose(2, 1, 0, 3)
    )

    lhs_all = []
    for c in range(NCORES):
        sl = slice(c * SH, (c + 1) * SH)
        lhs_aug = np.zeros((KT, P, SH), dtype=NP_F8)
        lhs_aug[:NKD] = np.ascontiguousarray(m1q[sl].T).reshape(NKD, P, SH)
        laug = np.zeros((P, SH), dtype=np.float32)
        laug[:NIDS] = -128.0 * onehot[:, sl].astype(np.float32)
        for t, cf in enumerate(COEFFS):
            laug[NIDS + t] = cf
        lhs_aug[NKD] = laug.astype(NP_F8)
        # pack [17,128,512] -> [128, 17, 512]
        lhs_p = np.ascontiguousarray(lhs_aug.transpose(1, 0, 2))
        lhs_all.append(lhs_p)
    return lhs_all, rhs_p, sq1


def run(modal1_inputs, modal2_inputs, targets, margin, trace=False):
    m1 = np.ascontiguousarray(np.asarray(modal1_inputs, dtype=np.float32))
    m2 = np.ascontiguousarray(np.asarray(modal2_inputs, dtype=np.float32))
    lhs_all, rhs_p, sq1 = _prep_host(m1, m2, targets)
    nc = _get_program()
    in_maps = [{"lhs": lhs_all[c], "rhs": rhs_p} for c in range(NCORES)]
    res = run_bass_kernel_spmd(nc, in_maps, list(range(NCORES)), trace=trace)

    # host finale: add norms, sqrt (monotone), hinge, precision
    pmin = np.empty(N)
    pmax = np.empty(N)
    for c in range(NCORES):
        o = np.asarray(res.results[c]["out"], dtype=np.float64)  # [P, 2*MT]
        sl = slice(c * SH, (c + 1) * SH)
        pmin[sl] = o[:, :MT].T.reshape(SH)
        pmax[sl] = o[:, MT:].T.reshape(SH)
    ap = np.sqrt(np.maximum(-2.0 * pmin + sq1 - BIG, EPS))
    an = np.sqrt(np.maximum(-2.0 * pmax + sq1, EPS))
    loss = np.float32(np.maximum(ap - an + float(margin), 0.0).mean())
    prec = np.float32((an > ap).mean())
    return (loss, prec), res


def kernel(modal1_inputs, modal2_inputs, targets, margin):
    (loss, prec), _ = run(modal1_inputs, modal2_inputs, targets, margin)
    return loss, prec
